# revision 12
# baseline (speedup 1.0000x reference)
"""Trainium2 Bass kernel for a 2-layer GCN + global mean pool + MLP head.

Strategy (8 NeuronCores, SPMD, one shared NEFF):
  - Nodes (= aggregation dsts) are sharded across cores: core c owns rows
    [c*12500, (c+1)*12500), padded to 12800 = 25 blocks of 512.
  - Layer 1 is computed as agg = A_norm @ x (gather + one-hot-matmul
    segment-sum on device), then h1e = elu(agg.T @ W1 + b1) per shard.
  - Layer 2 + global mean pool collapse algebraically: the pooled sums are
    (B^T A_norm) h1e W2 with B the graph one-hot; wmat = B^T A_norm is a
    [64 x N] matrix computed on host from edge_index alone (graph
    partitioning / preprocessing, per the sharding hint). Each core
    computes partial = wmat[:, shard].T-contraction with h1e on device.
  - Host epilogue: sum 8 partials, @W2, mean, fc1/relu/fc2, log_softmax
    (~0.003% of FLOPs).

  Non-loop edges are routed on host to fixed "slot tiles":
  run = (dst block of 512, src chunk of 25000), T_RUN=16 tiles of 128 edge
  slots each; tile j accumulates into PSUM columns [WBASE[j], WBASE[j]+128)
  of its block. The gathered 128-edge tile (bf16 rows of x, one 256B
  descriptor per edge) is the matmul stationary operand; the moving operand
  is a host-precomputed sparse routing tile oh[slot, w] = norm_e for the
  edge in that slot (norm folded in, zero elsewhere), streamed as bf16.
  All instruction structure (incl. PSUM offsets) is input-independent, so
  the NEFF is reusable for any same-shape input.

  Self loops bypass the gather entirely: their contribution is
  diag(dinv^2) applied to the core's own dense x shard, computed as NT
  small diagonal matmuls per block (xsh is a per-core sharded input).

  Overflow valve: edges that don't fit their run's slots (~1% at 97.7%
  slot occupancy) are pre-aggregated on host into aggX, which also
  serves as the PSUM initializer via an identity-matmul.

  Perf note: the kernel is bound by SWDGE descriptor generation for the
  edge gather (~8 ns/descriptor on GpSimd Q7 cores 0/1, measured; engine-
  serial). DMA, TensorE and DVE all hide underneath it. dma_gather is the
  fastest indexed-move primitive on TRN2 (ap_gather measured ~33 ns/idx,
  single_packet=True and fp32 gathers hang the device, host-side
  descriptor generation only exists for compile-time-static patterns), so
  minimizing descriptor count (wide 128-dst windows, self-loop bypass,
  tight 97.7% slot packing) is the whole game.
"""
import sys
import numpy as np

sys.path.insert(0, "/opt/trn_rl_repo")


# ---------------------------------------------------------------- config ----
class CFG:
    def __init__(self, N=100000, D=128, G=64, n_cores=8, n_chunk=4, blk=512,
                 t_run=16, w=128, stride=27):
        self.N, self.D, self.G = N, D, G
        self.N_CORES, self.N_CHUNK, self.BLK = n_cores, n_chunk, blk
        self.T_RUN, self.W, self.STRIDE = t_run, w, stride
        assert N % n_cores == 0 and N % n_chunk == 0
        self.SHARD = N // n_cores
        self.CHUNK = N // n_chunk
        assert self.CHUNK <= 32768, "src chunk ids must fit int16"
        self.N_BLK = (self.SHARD + blk - 1) // blk
        self.SHARD_PAD = self.N_BLK * blk
        self.WBASE = np.minimum(np.arange(t_run) * stride, blk - w)
        assert self.WBASE[-1] == blk - w, "windows must cover the block"
        assert np.all(np.diff(self.WBASE) <= w)
        self.SLOTS = t_run * 128
        self.N_RUN = self.N_BLK * n_chunk
        self.NT = blk // 128  # node tiles per block


FULL = CFG()


# ---------------------------------------------------------- preprocessing ----
def preprocess(cfg, x, edge_index, batch):
    """Host-side graph partitioning: per-core gather/routing streams."""
    src = np.asarray(edge_index[0], dtype=np.int64)
    dst = np.asarray(edge_index[1], dtype=np.int64)
    batch = np.asarray(batch, dtype=np.int64)
    N, G = cfg.N, cfg.G

    deg = np.bincount(dst, minlength=N).astype(np.float32) + 1.0  # + self loop
    dinv = (1.0 / np.sqrt(deg)).astype(np.float32)

    loops = np.arange(N, dtype=np.int64)
    src_wm = np.concatenate([src, loops])
    dst_wm = np.concatenate([dst, loops])
    norm_wm = (dinv[src_wm] * dinv[dst_wm]).astype(np.float32)

    flat = batch[dst_wm] * N + src_wm
    wmat = np.bincount(flat, weights=norm_wm.astype(np.float64),
                       minlength=G * N).reshape(G, N).astype(np.float32)

    # self loops bypass the gather (handled by diag matmuls on dense x)
    src_all, dst_all, norm_all = src, dst, norm_wm[:len(src)]
    dl2 = (dinv * dinv).astype(np.float32)  # self-loop weights

    wb, W, T_RUN, BLK = cfg.WBASE, cfg.W, cfg.T_RUN, cfg.BLK
    per_core = []
    n_overflow = 0
    for c in range(cfg.N_CORES):
        m = (dst_all >= c * cfg.SHARD) & (dst_all < (c + 1) * cfg.SHARD)
        s_c = src_all[m]
        dl_c = dst_all[m] - c * cfg.SHARD
        nv_c = norm_all[m]
        blk = dl_c // BLK
        chunk = s_c // cfg.CHUNK
        o = np.lexsort((dl_c, chunk, blk))
        s_c, dl_c, nv_c, blk, chunk = s_c[o], dl_c[o], nv_c[o], blk[o], chunk[o]

        gidx = np.zeros((cfg.N_RUN, cfg.SLOTS), dtype=np.int16)
        # routing one-hot with norm folded in: [run, slot, W]
        ohv = np.zeros((cfg.N_RUN, cfg.SLOTS, W), dtype=np.float32)
        aggX = np.zeros((cfg.SHARD_PAD, cfg.D), dtype=np.float32)
        ovf_d, ovf_s, ovf_n = [], [], []

        run_id = blk * cfg.N_CHUNK + chunk
        run_starts = np.searchsorted(run_id, np.arange(cfg.N_RUN))
        run_ends = np.searchsorted(run_id, np.arange(cfg.N_RUN) + 1)
        for r in range(cfg.N_RUN):
            a, b = int(run_starts[r]), int(run_ends[r])
            if a == b:
                continue
            bb = (r // cfg.N_CHUNK) * BLK
            ch = r % cfg.N_CHUNK
            drel = dl_c[a:b] - bb
            # exact greedy: leftmost eligible tile, ascending dst
            starts = np.searchsorted(drel, wb)
            ends = np.searchsorted(drel, wb + W)
            tile_of = np.full(b - a, -1, dtype=np.int64)
            placed = 0
            for j in range(T_RUN):
                lo = max(placed, int(starts[j]))
                take = min(128, int(ends[j]) - lo)
                if take > 0:
                    tile_of[lo:lo + take] = j
                    placed = lo + take
                else:
                    placed = max(placed, lo)
            okm = tile_of >= 0
            tloc = tile_of[okm]
            loads = np.bincount(tloc, minlength=T_RUN)
            cum = np.concatenate([[0], np.cumsum(loads)])[:-1]
            pos = np.arange(len(tloc)) - np.repeat(cum, loads)
            slot = tloc * 128 + pos
            gidx[r, slot] = (s_c[a:b][okm] - ch * cfg.CHUNK).astype(np.int16)
            ohv[r, slot, (drel[okm] - wb[tloc]).astype(np.int64)] = nv_c[a:b][okm]
            ovf = np.nonzero(~okm)[0]
            if len(ovf):
                n_overflow += len(ovf)
                ovf_d.append(dl_c[a + ovf])
                ovf_s.append(s_c[a + ovf])
                ovf_n.append(nv_c[a + ovf])

        if ovf_d:
            od = np.concatenate(ovf_d)
            osrc = np.concatenate(ovf_s)
            on = np.concatenate(ovf_n)
            np.add.at(aggX, od,
                      on[:, None] * np.asarray(x, dtype=np.float32)[osrc])

        # self-loop diag tiles: dg[b, p, nt, q] = dl2[node] at p == q
        sh0 = c * cfg.SHARD
        dlpad = np.zeros(cfg.SHARD_PAD, dtype=np.float32)
        dlpad[:cfg.SHARD] = dl2[sh0:sh0 + cfg.SHARD]
        dg = np.zeros((cfg.N_BLK, 128, cfg.NT, 128), dtype=np.float32)
        ar = np.arange(128)
        dgv = dlpad.reshape(cfg.N_BLK, cfg.NT, 128)
        for b in range(cfg.N_BLK):
            for nt in range(cfg.NT):
                dg[b, ar, nt, ar] = dgv[b, nt]

        wT = np.zeros((cfg.SHARD_PAD, G), dtype=np.float32)
        wT[:cfg.SHARD] = wmat[:, c * cfg.SHARD:(c + 1) * cfg.SHARD].T

        import ml_dtypes
        per_core.append({
            # wrapped [16, SLOTS//16] then replicated to 128 partitions
            # (the SWDGE desc-gen cores read 16-partition wrapped indices)
            "gidx": np.ascontiguousarray(np.tile(
                gidx.reshape(cfg.N_RUN, cfg.SLOTS // 16, 16)
                    .transpose(0, 2, 1), (1, 8, 1))),
            # [run, 128 slot-partitions, T, W]
            "oh": np.ascontiguousarray(
                ohv.reshape(cfg.N_RUN, T_RUN, 128, W).transpose(0, 2, 1, 3)
            ).astype(ml_dtypes.bfloat16),
            "aggX": np.ascontiguousarray(aggX.T),      # [D, SHARD_PAD]
            "dg": dg.astype(ml_dtypes.bfloat16),        # [N_BLK, 128, NT, 128]
            "wT": wT,                                   # [SHARD_PAD, G]
        })

    cnt = np.bincount(batch, minlength=G).astype(np.float32)
    return per_core, cnt, n_overflow


def pack_x(x):
    """[N, D] f32 -> [N, D] bf16."""
    import ml_dtypes
    return np.ascontiguousarray(
        np.asarray(x, dtype=np.float32).astype(ml_dtypes.bfloat16))


# ---------------------------------------------------------- bass kernel ----
def build_kernel(cfg):
    from concourse import bacc, bass, tile, mybir
    from concourse.masks import make_identity
    f32 = mybir.dt.float32
    bf16 = mybir.dt.bfloat16

    nc = bacc.Bacc("TRN2", target_bir_lowering=False, debug=False,
                   enable_asserts=False)
    x_t = nc.dram_tensor("x", [cfg.N, cfg.D], bf16, kind="ExternalInput")
    xsh_t = nc.dram_tensor("xsh", [cfg.SHARD_PAD, cfg.D], bf16,
                           kind="ExternalInput")
    gidx_t = nc.dram_tensor("gidx", [cfg.N_RUN, 128, cfg.SLOTS // 16],
                            mybir.dt.int16, kind="ExternalInput")
    oh_t = nc.dram_tensor("oh", [cfg.N_RUN, 128, cfg.T_RUN, cfg.W], bf16,
                          kind="ExternalInput")
    aggx_t = nc.dram_tensor("aggX", [cfg.D, cfg.SHARD_PAD], bf16,
                            kind="ExternalInput")
    dg_t = nc.dram_tensor("dg", [cfg.N_BLK, 128, cfg.NT, 128], bf16,
                          kind="ExternalInput")
    wT_t = nc.dram_tensor("wT", [cfg.SHARD_PAD, cfg.G], f32,
                          kind="ExternalInput")
    W1_t = nc.dram_tensor("W1", [cfg.D, cfg.D], f32, kind="ExternalInput")
    b1_t = nc.dram_tensor("b1", [1, cfg.D], f32, kind="ExternalInput")
    out_t = nc.dram_tensor("partial", [cfg.G, cfg.D], f32,
                           kind="ExternalOutput")

    T, W, NT, BLK = cfg.T_RUN, cfg.W, cfg.NT, cfg.BLK
    WB = [int(v) for v in cfg.WBASE]
    mult, add, amax = (mybir.AluOpType.mult, mybir.AluOpType.add,
                       mybir.AluOpType.max)

    with tile.TileContext(nc) as tc:
        with (tc.tile_pool(name="const", bufs=1) as cpool,
              tc.tile_pool(name="gbuf", bufs=3) as gpool,
              tc.tile_pool(name="meta", bufs=4) as mpool,
              tc.tile_pool(name="oh", bufs=3) as opool,
              tc.tile_pool(name="agg", bufs=2) as apool,
              tc.tile_pool(name="eluv", bufs=2) as epool,
              tc.tile_pool(name="wt", bufs=2) as wpool,
              tc.tile_pool(name="ax", bufs=2) as axpool,
              tc.tile_pool(name="psA", bufs=2, space="PSUM") as pApool,
              tc.tile_pool(name="psB", bufs=2, space="PSUM") as pBpool,
              tc.tile_pool(name="psC", bufs=1, space="PSUM") as pCpool,
              tc.tile_pool(name="outp", bufs=1) as outpool):

            ident = cpool.tile([128, 128], bf16)
            make_identity(nc, ident[:])
            W1s = cpool.tile([128, cfg.D], f32)
            nc.sync.dma_start(W1s[:], W1_t.ap())
            b1s = cpool.tile([128, cfg.D], f32)
            nc.sync.dma_start(b1s[:], b1_t.ap().to_broadcast((128, cfg.D)))

            psC = pCpool.tile([cfg.G, cfg.D], f32)

            for b in range(cfg.N_BLK):
                axs = axpool.tile([128, BLK], bf16)
                nc.sync.dma_start(axs[:], aggx_t.ap()[:, b * BLK:(b + 1) * BLK])
                psA = pApool.tile([128, BLK], f32)
                nc.tensor.matmul(out=psA[:], lhsT=ident[:], rhs=axs[:],
                                 start=True, stop=False)

                # self loops: psA[:, g] += xsh_g.T @ diag(dinv^2)
                xds = axpool.tile([128, cfg.NT, cfg.D], bf16, tag="xd")
                nc.sync.dma_start(
                    xds[:],
                    xsh_t.ap()[b * BLK:(b + 1) * BLK, :]
                        .rearrange("(t p) d -> p t d", p=128))
                dgs = axpool.tile([128, cfg.NT, 128], bf16, tag="dg")
                nc.sync.dma_start(dgs[:], dg_t.ap()[b])
                for nt in range(NT):
                    nc.tensor.matmul(
                        out=psA[:, nt * 128:(nt + 1) * 128],
                        lhsT=xds[:, nt, :], rhs=dgs[:, nt, :],
                        start=False, stop=False)
                for ch in range(cfg.N_CHUNK):
                    r = b * cfg.N_CHUNK + ch
                    gxs = mpool.tile([128, cfg.SLOTS // 16], mybir.dt.int16,
                                     tag="gx")
                    nc.sync.dma_start(gxs[:], gidx_t.ap()[r])
                    oh = opool.tile([128, T, W], bf16)
                    nc.sync.dma_start(oh[:], oh_t.ap()[r])

                    gb = gpool.tile([128, T, cfg.D], bf16)
                    nc.gpsimd.dma_gather(
                        out_ap=gb[:],
                        in_ap=x_t.ap()[ch * cfg.CHUNK:(ch + 1) * cfg.CHUNK, :],
                        idxs_ap=gxs[:],
                        num_idxs=cfg.SLOTS,
                        num_idxs_reg=cfg.SLOTS,
                        elem_size=cfg.D,
                        single_packet=False,
                    )

                    for t in range(T):
                        last = (ch == cfg.N_CHUNK - 1 and t == T - 1)
                        nc.tensor.matmul(
                            out=psA[:, WB[t]:WB[t] + W],
                            lhsT=gb[:, t, :],
                            rhs=oh[:, t, :],
                            start=False, stop=last,
                        )

                # drain agg (feat-major [D x BLK])
                aggs = apool.tile([128, BLK], f32)
                nc.vector.tensor_copy(out=aggs[:], in_=psA[:])

                # B: h1 = agg.T @ W1  -> psB [node x feat_out], per node tile
                psB = pBpool.tile([128, BLK], f32)
                for nt in range(NT):
                    nc.tensor.matmul(out=psB[:, nt * cfg.D:(nt + 1) * cfg.D],
                                     lhsT=aggs[:, nt * 128:(nt + 1) * 128],
                                     rhs=W1s[:], start=True, stop=True)

                # elu(x+b1) = max(xb, min(exp(xb)-1, 0))
                xb = epool.tile([128, NT, cfg.D], f32, tag="xb")
                nc.vector.tensor_tensor(
                    out=xb[:],
                    in0=psB[:].rearrange("p (t d) -> p t d", d=cfg.D),
                    in1=b1s[:].unsqueeze(1).broadcast_to((128, NT, cfg.D)),
                    op=add)
                ex = epool.tile([128, NT * cfg.D], f32, tag="ex")
                nc.scalar.activation(
                    out=ex[:], in_=xb[:].rearrange("p t d -> p (t d)"),
                    func=mybir.ActivationFunctionType.Exp)
                nc.vector.tensor_scalar(
                    out=ex[:], in0=ex[:], scalar1=-1.0, scalar2=0.0,
                    op0=add, op1=mybir.AluOpType.min)
                h1e = epool.tile([128, NT * cfg.D], f32, tag="h1e")
                nc.vector.tensor_tensor(
                    out=h1e[:], in0=xb[:].rearrange("p t d -> p (t d)"),
                    in1=ex[:], op=amax)

                # C: partial += wT_block.T @ h1e
                wts = wpool.tile([128, NT, cfg.G], f32)
                nc.sync.dma_start(
                    wts[:],
                    wT_t.ap()[b * BLK:(b + 1) * BLK, :]
                        .rearrange("(t p) g -> p t g", p=128))
                for nt in range(NT):
                    nc.tensor.matmul(
                        out=psC[:],
                        lhsT=wts[:, nt, :],
                        rhs=h1e[:, nt * cfg.D:(nt + 1) * cfg.D],
                        start=(b == 0 and nt == 0),
                        stop=(b == cfg.N_BLK - 1 and nt == NT - 1),
                    )

            outs = outpool.tile([cfg.G, cfg.D], f32)
            nc.vector.tensor_copy(out=outs[:], in_=psC[:])
            nc.sync.dma_start(out_t.ap(), outs[:])

    nc.compile()
    return nc


# ------------------------------------------------------------- epilogue ----
def epilogue(partials, cnt, W2, b2, fc1_W, fc1_b, fc2_W, fc2_b):
    g_sum = np.sum(partials, axis=0, dtype=np.float32)
    S = g_sum @ W2 + cnt[:, None] * b2[None, :]
    mean = S / np.maximum(cnt, 1.0)[:, None]
    z = np.maximum(mean @ fc1_W + fc1_b[None, :], 0.0)
    z = z @ fc2_W + fc2_b[None, :]
    zmax = z.max(axis=1, keepdims=True)
    lse = np.log(np.sum(np.exp(z - zmax), axis=1, keepdims=True)) + zmax
    return (z - lse).astype(np.float32)


_NC_CACHE = {}


def run_on_device(cfg, per_core, x, trace=False):
    key = (cfg.N, cfg.D, cfg.G, cfg.N_CORES, cfg.T_RUN, cfg.W)
    if key not in _NC_CACHE:
        _NC_CACHE[key] = build_kernel(cfg)
    nc = _NC_CACHE[key]
    import ml_dtypes
    xp = pack_x(x)
    b1z = np.zeros((1, cfg.D), dtype=np.float32)
    in_maps = []
    for c in range(cfg.N_CORES):
        s = per_core[c]
        xsh = np.zeros((cfg.SHARD_PAD, cfg.D), dtype=xp.dtype)
        xsh[:cfg.SHARD] = xp[c * cfg.SHARD:(c + 1) * cfg.SHARD]
        in_maps.append({
            "x": xp, "xsh": xsh, "gidx": s["gidx"], "oh": s["oh"],
            "aggX": s["aggX"].astype(ml_dtypes.bfloat16), "dg": s["dg"],
            "wT": s["wT"], "W1": None, "b1": b1z,
        })
    return nc, in_maps


def kernel(x, edge_index, batch, W1, b1, W2, b2, fc1_W, fc1_b, fc2_W, fc2_b):
    from concourse import bass_utils
    cfg = FULL
    per_core, cnt, _ = preprocess(cfg, x, edge_index, batch)
    nc, in_maps = run_on_device(cfg, per_core, x)
    W1f = np.ascontiguousarray(np.asarray(W1, dtype=np.float32))
    b1f = np.asarray(b1, dtype=np.float32).reshape(1, cfg.D)
    for m in in_maps:
        m["W1"] = W1f
        m["b1"] = b1f
    res = bass_utils.run_bass_kernel_spmd(
        nc, in_maps, core_ids=list(range(cfg.N_CORES)))
    partials = [res.results[c]["partial"] for c in range(cfg.N_CORES)]
    out = epilogue(partials, cnt,
                   np.asarray(W2, np.float32), np.asarray(b2, np.float32),
                   np.asarray(fc1_W, np.float32), np.asarray(fc1_b, np.float32),
                   np.asarray(fc2_W, np.float32), np.asarray(fc2_b, np.float32))
    return out


# revision 14
# speedup vs baseline: 1.0606x; 1.0606x over previous
"""Trainium2 Bass kernel for a 2-layer GCN + global mean pool + MLP head.

Strategy (8 NeuronCores, SPMD, one shared NEFF):
  - Nodes (= aggregation dsts) are sharded across cores: core c owns rows
    [c*12500, (c+1)*12500), padded to 12800 = 25 blocks of 512.
  - Layer 1 is computed as agg = A_norm @ x (gather + one-hot-matmul
    segment-sum on device), then h1e = elu(agg.T @ W1 + b1) per shard.
  - Layer 2 + global mean pool collapse algebraically: the pooled sums are
    (B^T A_norm) h1e W2 with B the graph one-hot; wmat = B^T A_norm is a
    [64 x N] matrix computed on host from edge_index alone (graph
    partitioning / preprocessing, per the sharding hint). Each core
    computes partial = wmat[:, shard].T-contraction with h1e on device.
  - Host epilogue: sum 8 partials, @W2, mean, fc1/relu/fc2, log_softmax
    (~0.003% of FLOPs).

  Non-loop edges are routed on host to fixed "slot tiles":
  run = (dst block of 512, src chunk of 25000), T_RUN=16 tiles of 128 edge
  slots each; tile j accumulates into PSUM columns [WBASE[j], WBASE[j]+128)
  of its block. The gathered 128-edge tile (bf16 rows of x, one 256B
  descriptor per edge) is the matmul stationary operand; the moving operand
  is a host-precomputed sparse routing tile oh[slot, w] = norm_e for the
  edge in that slot (norm folded in, zero elsewhere), streamed as bf16.
  All instruction structure (incl. PSUM offsets) is input-independent, so
  the NEFF is reusable for any same-shape input.

  Self loops bypass the gather entirely: their contribution is
  diag(dinv^2) applied to the core's own dense x shard, computed as NT
  small diagonal matmuls per block (xsh is a per-core sharded input).

  Overflow valve: edges that don't fit their run's slots (~1% at 97.7%
  slot occupancy) are pre-aggregated on host into aggX, which also
  serves as the PSUM initializer via an identity-matmul.

  Perf note: the kernel is bound by SWDGE descriptor generation for the
  edge gather (~8 ns/descriptor on GpSimd Q7 cores 0/1, measured; engine-
  serial). DMA, TensorE and DVE all hide underneath it. dma_gather is the
  fastest indexed-move primitive on TRN2 (ap_gather measured ~33 ns/idx,
  single_packet=True and fp32 gathers hang the device, host-side
  descriptor generation only exists for compile-time-static patterns), so
  minimizing descriptor count (wide 128-dst windows, self-loop bypass,
  tight 97.7% slot packing) is the whole game.
"""
import sys
import numpy as np

sys.path.insert(0, "/opt/trn_rl_repo")


# ---------------------------------------------------------------- config ----
class CFG:
    def __init__(self, N=100000, D=128, G=64, n_cores=8, n_chunk=4, blk=512,
                 t_run=15, w=192, stride=23):
        self.N, self.D, self.G = N, D, G
        self.N_CORES, self.N_CHUNK, self.BLK = n_cores, n_chunk, blk
        self.T_RUN, self.W, self.STRIDE = t_run, w, stride
        assert N % n_cores == 0 and N % n_chunk == 0
        self.SHARD = N // n_cores
        self.CHUNK = N // n_chunk
        assert self.CHUNK <= 32768, "src chunk ids must fit int16"
        self.N_BLK = (self.SHARD + blk - 1) // blk
        self.SHARD_PAD = self.N_BLK * blk
        self.WBASE = np.minimum(np.arange(t_run) * stride, blk - w)
        assert self.WBASE[-1] == blk - w, "windows must cover the block"
        assert np.all(np.diff(self.WBASE) <= w)
        self.SLOTS = t_run * 128
        self.N_RUN = self.N_BLK * n_chunk
        self.NT = blk // 128  # node tiles per block


FULL = CFG()


# ---------------------------------------------------------- preprocessing ----
def preprocess(cfg, x, edge_index, batch):
    """Host-side graph partitioning: per-core gather/routing streams."""
    src = np.asarray(edge_index[0], dtype=np.int64)
    dst = np.asarray(edge_index[1], dtype=np.int64)
    batch = np.asarray(batch, dtype=np.int64)
    N, G = cfg.N, cfg.G

    deg = np.bincount(dst, minlength=N).astype(np.float32) + 1.0  # + self loop
    dinv = (1.0 / np.sqrt(deg)).astype(np.float32)

    loops = np.arange(N, dtype=np.int64)
    src_wm = np.concatenate([src, loops])
    dst_wm = np.concatenate([dst, loops])
    norm_wm = (dinv[src_wm] * dinv[dst_wm]).astype(np.float32)

    flat = batch[dst_wm] * N + src_wm
    wmat = np.bincount(flat, weights=norm_wm.astype(np.float64),
                       minlength=G * N).reshape(G, N).astype(np.float32)

    # self loops bypass the gather (handled by diag matmuls on dense x)
    src_all, dst_all, norm_all = src, dst, norm_wm[:len(src)]
    dl2 = (dinv * dinv).astype(np.float32)  # self-loop weights

    wb, W, T_RUN, BLK = cfg.WBASE, cfg.W, cfg.T_RUN, cfg.BLK
    per_core = []
    n_overflow = 0
    for c in range(cfg.N_CORES):
        m = (dst_all >= c * cfg.SHARD) & (dst_all < (c + 1) * cfg.SHARD)
        s_c = src_all[m]
        dl_c = dst_all[m] - c * cfg.SHARD
        nv_c = norm_all[m]
        blk = dl_c // BLK
        chunk = s_c // cfg.CHUNK
        o = np.lexsort((dl_c, chunk, blk))
        s_c, dl_c, nv_c, blk, chunk = s_c[o], dl_c[o], nv_c[o], blk[o], chunk[o]

        gidx = np.zeros((cfg.N_RUN, cfg.SLOTS), dtype=np.int16)
        # routing one-hot with norm folded in: [run, slot, W]
        ohv = np.zeros((cfg.N_RUN, cfg.SLOTS, W), dtype=np.float32)
        aggX = np.zeros((cfg.SHARD_PAD, cfg.D), dtype=np.float32)
        ovf_d, ovf_s, ovf_n = [], [], []

        run_id = blk * cfg.N_CHUNK + chunk
        run_starts = np.searchsorted(run_id, np.arange(cfg.N_RUN))
        run_ends = np.searchsorted(run_id, np.arange(cfg.N_RUN) + 1)
        for r in range(cfg.N_RUN):
            a, b = int(run_starts[r]), int(run_ends[r])
            if a == b:
                continue
            bb = (r // cfg.N_CHUNK) * BLK
            ch = r % cfg.N_CHUNK
            drel = dl_c[a:b] - bb
            # exact greedy: leftmost eligible tile, ascending dst
            starts = np.searchsorted(drel, wb)
            ends = np.searchsorted(drel, wb + W)
            tile_of = np.full(b - a, -1, dtype=np.int64)
            placed = 0
            for j in range(T_RUN):
                lo = max(placed, int(starts[j]))
                take = min(128, int(ends[j]) - lo)
                if take > 0:
                    tile_of[lo:lo + take] = j
                    placed = lo + take
                else:
                    placed = max(placed, lo)
            okm = tile_of >= 0
            tloc = tile_of[okm]
            loads = np.bincount(tloc, minlength=T_RUN)
            cum = np.concatenate([[0], np.cumsum(loads)])[:-1]
            pos = np.arange(len(tloc)) - np.repeat(cum, loads)
            slot = tloc * 128 + pos
            gidx[r, slot] = (s_c[a:b][okm] - ch * cfg.CHUNK).astype(np.int16)
            ohv[r, slot, (drel[okm] - wb[tloc]).astype(np.int64)] = nv_c[a:b][okm]
            ovf = np.nonzero(~okm)[0]
            if len(ovf):
                n_overflow += len(ovf)
                ovf_d.append(dl_c[a + ovf])
                ovf_s.append(s_c[a + ovf])
                ovf_n.append(nv_c[a + ovf])

        if ovf_d:
            od = np.concatenate(ovf_d)
            osrc = np.concatenate(ovf_s)
            on = np.concatenate(ovf_n)
            np.add.at(aggX, od,
                      on[:, None] * np.asarray(x, dtype=np.float32)[osrc])

        # self-loop diag tiles: dg[b, p, nt, q] = dl2[node] at p == q
        sh0 = c * cfg.SHARD
        dlpad = np.zeros(cfg.SHARD_PAD, dtype=np.float32)
        dlpad[:cfg.SHARD] = dl2[sh0:sh0 + cfg.SHARD]
        dg = np.zeros((cfg.N_BLK, 128, cfg.NT, 128), dtype=np.float32)
        ar = np.arange(128)
        dgv = dlpad.reshape(cfg.N_BLK, cfg.NT, 128)
        for b in range(cfg.N_BLK):
            for nt in range(cfg.NT):
                dg[b, ar, nt, ar] = dgv[b, nt]

        wT = np.zeros((cfg.SHARD_PAD, G), dtype=np.float32)
        wT[:cfg.SHARD] = wmat[:, c * cfg.SHARD:(c + 1) * cfg.SHARD].T

        import ml_dtypes
        per_core.append({
            # wrapped [16, SLOTS//16] then replicated to 128 partitions
            # (the SWDGE desc-gen cores read 16-partition wrapped indices)
            "gidx": np.ascontiguousarray(np.tile(
                gidx.reshape(cfg.N_RUN, cfg.SLOTS // 16, 16)
                    .transpose(0, 2, 1), (1, 8, 1))),
            # [run, 128 slot-partitions, T, W]
            "oh": np.ascontiguousarray(
                ohv.reshape(cfg.N_RUN, T_RUN, 128, W).transpose(0, 2, 1, 3)
            ).astype(ml_dtypes.bfloat16),
            "aggX": np.ascontiguousarray(aggX.T),      # [D, SHARD_PAD]
            "dg": dg.astype(ml_dtypes.bfloat16),        # [N_BLK, 128, NT, 128]
            "wT": wT,                                   # [SHARD_PAD, G]
        })

    cnt = np.bincount(batch, minlength=G).astype(np.float32)
    return per_core, cnt, n_overflow


def pack_x(x):
    """[N, D] f32 -> [N, D] bf16."""
    import ml_dtypes
    return np.ascontiguousarray(
        np.asarray(x, dtype=np.float32).astype(ml_dtypes.bfloat16))


# ---------------------------------------------------------- bass kernel ----
def build_kernel(cfg):
    from concourse import bacc, bass, tile, mybir
    from concourse.masks import make_identity
    f32 = mybir.dt.float32
    bf16 = mybir.dt.bfloat16

    nc = bacc.Bacc("TRN2", target_bir_lowering=False, debug=False,
                   enable_asserts=False)
    x_t = nc.dram_tensor("x", [cfg.N, cfg.D], bf16, kind="ExternalInput")
    xsh_t = nc.dram_tensor("xsh", [cfg.SHARD_PAD, cfg.D], bf16,
                           kind="ExternalInput")
    gidx_t = nc.dram_tensor("gidx", [cfg.N_RUN, 128, cfg.SLOTS // 16],
                            mybir.dt.int16, kind="ExternalInput")
    oh_t = nc.dram_tensor("oh", [cfg.N_RUN, 128, cfg.T_RUN, cfg.W], bf16,
                          kind="ExternalInput")
    aggx_t = nc.dram_tensor("aggX", [cfg.D, cfg.SHARD_PAD], bf16,
                            kind="ExternalInput")
    dg_t = nc.dram_tensor("dg", [cfg.N_BLK, 128, cfg.NT, 128], bf16,
                          kind="ExternalInput")
    wT_t = nc.dram_tensor("wT", [cfg.SHARD_PAD, cfg.G], f32,
                          kind="ExternalInput")
    W1_t = nc.dram_tensor("W1", [cfg.D, cfg.D], f32, kind="ExternalInput")
    b1_t = nc.dram_tensor("b1", [1, cfg.D], f32, kind="ExternalInput")
    out_t = nc.dram_tensor("partial", [cfg.G, cfg.D], f32,
                           kind="ExternalOutput")

    T, W, NT, BLK = cfg.T_RUN, cfg.W, cfg.NT, cfg.BLK
    WB = [int(v) for v in cfg.WBASE]
    mult, add, amax = (mybir.AluOpType.mult, mybir.AluOpType.add,
                       mybir.AluOpType.max)

    with tile.TileContext(nc) as tc:
        with (tc.tile_pool(name="const", bufs=1) as cpool,
              tc.tile_pool(name="gbuf", bufs=4) as gpool,
              tc.tile_pool(name="meta", bufs=6) as mpool,
              tc.tile_pool(name="oh", bufs=4) as opool,
              tc.tile_pool(name="agg", bufs=2) as apool,
              tc.tile_pool(name="eluv", bufs=2) as epool,
              tc.tile_pool(name="wt", bufs=2) as wpool,
              tc.tile_pool(name="ax", bufs=2) as axpool,
              tc.tile_pool(name="psA", bufs=2, space="PSUM") as pApool,
              tc.tile_pool(name="psB", bufs=2, space="PSUM") as pBpool,
              tc.tile_pool(name="psC", bufs=1, space="PSUM") as pCpool,
              tc.tile_pool(name="outp", bufs=1) as outpool):

            ident = cpool.tile([128, 128], bf16)
            make_identity(nc, ident[:])
            W1s = cpool.tile([128, cfg.D], f32)
            nc.sync.dma_start(W1s[:], W1_t.ap())
            b1s = cpool.tile([128, cfg.D], f32)
            nc.sync.dma_start(b1s[:], b1_t.ap().to_broadcast((128, cfg.D)))

            psC = pCpool.tile([cfg.G, cfg.D], f32)

            for b in range(cfg.N_BLK):
                axs = axpool.tile([128, BLK], bf16)
                nc.sync.dma_start(axs[:], aggx_t.ap()[:, b * BLK:(b + 1) * BLK])
                psA = pApool.tile([128, BLK], f32)
                nc.tensor.matmul(out=psA[:], lhsT=ident[:], rhs=axs[:],
                                 start=True, stop=False)

                # self loops: psA[:, g] += xsh_g.T @ diag(dinv^2)
                xds = axpool.tile([128, cfg.NT, cfg.D], bf16, tag="xd")
                nc.sync.dma_start(
                    xds[:],
                    xsh_t.ap()[b * BLK:(b + 1) * BLK, :]
                        .rearrange("(t p) d -> p t d", p=128))
                dgs = axpool.tile([128, cfg.NT, 128], bf16, tag="dg")
                nc.sync.dma_start(dgs[:], dg_t.ap()[b])
                for nt in range(NT):
                    nc.tensor.matmul(
                        out=psA[:, nt * 128:(nt + 1) * 128],
                        lhsT=xds[:, nt, :], rhs=dgs[:, nt, :],
                        start=False, stop=False)
                for ch in range(cfg.N_CHUNK):
                    r = b * cfg.N_CHUNK + ch
                    gxs = mpool.tile([128, cfg.SLOTS // 16], mybir.dt.int16,
                                     tag="gx")
                    nc.sync.dma_start(gxs[:], gidx_t.ap()[r])
                    oh = opool.tile([128, T, W], bf16)
                    nc.sync.dma_start(oh[:], oh_t.ap()[r])

                    gb = gpool.tile([128, T, cfg.D], bf16)
                    nc.gpsimd.dma_gather(
                        out_ap=gb[:],
                        in_ap=x_t.ap()[ch * cfg.CHUNK:(ch + 1) * cfg.CHUNK, :],
                        idxs_ap=gxs[:],
                        num_idxs=cfg.SLOTS,
                        num_idxs_reg=cfg.SLOTS,
                        elem_size=cfg.D,
                        single_packet=False,
                    )

                    for t in range(T):
                        last = (ch == cfg.N_CHUNK - 1 and t == T - 1)
                        nc.tensor.matmul(
                            out=psA[:, WB[t]:WB[t] + W],
                            lhsT=gb[:, t, :],
                            rhs=oh[:, t, :],
                            start=False, stop=last,
                        )

                # drain agg (feat-major [D x BLK])
                aggs = apool.tile([128, BLK], f32)
                nc.vector.tensor_copy(out=aggs[:], in_=psA[:])

                # B: h1 = agg.T @ W1  -> psB [node x feat_out], per node tile
                psB = pBpool.tile([128, BLK], f32)
                for nt in range(NT):
                    nc.tensor.matmul(out=psB[:, nt * cfg.D:(nt + 1) * cfg.D],
                                     lhsT=aggs[:, nt * 128:(nt + 1) * 128],
                                     rhs=W1s[:], start=True, stop=True)

                # elu(x+b1) = max(xb, min(exp(xb)-1, 0))
                xb = epool.tile([128, NT, cfg.D], f32, tag="xb")
                nc.vector.tensor_tensor(
                    out=xb[:],
                    in0=psB[:].rearrange("p (t d) -> p t d", d=cfg.D),
                    in1=b1s[:].unsqueeze(1).broadcast_to((128, NT, cfg.D)),
                    op=add)
                ex = epool.tile([128, NT * cfg.D], f32, tag="ex")
                nc.scalar.activation(
                    out=ex[:], in_=xb[:].rearrange("p t d -> p (t d)"),
                    func=mybir.ActivationFunctionType.Exp)
                nc.vector.tensor_scalar(
                    out=ex[:], in0=ex[:], scalar1=-1.0, scalar2=0.0,
                    op0=add, op1=mybir.AluOpType.min)
                h1e = epool.tile([128, NT * cfg.D], f32, tag="h1e")
                nc.vector.tensor_tensor(
                    out=h1e[:], in0=xb[:].rearrange("p t d -> p (t d)"),
                    in1=ex[:], op=amax)

                # C: partial += wT_block.T @ h1e
                wts = wpool.tile([128, NT, cfg.G], f32)
                nc.sync.dma_start(
                    wts[:],
                    wT_t.ap()[b * BLK:(b + 1) * BLK, :]
                        .rearrange("(t p) g -> p t g", p=128))
                for nt in range(NT):
                    nc.tensor.matmul(
                        out=psC[:],
                        lhsT=wts[:, nt, :],
                        rhs=h1e[:, nt * cfg.D:(nt + 1) * cfg.D],
                        start=(b == 0 and nt == 0),
                        stop=(b == cfg.N_BLK - 1 and nt == NT - 1),
                    )

            outs = outpool.tile([cfg.G, cfg.D], f32)
            nc.vector.tensor_copy(out=outs[:], in_=psC[:])
            nc.sync.dma_start(out_t.ap(), outs[:])

    nc.compile()
    return nc


# ------------------------------------------------------------- epilogue ----
def epilogue(partials, cnt, W2, b2, fc1_W, fc1_b, fc2_W, fc2_b):
    g_sum = np.sum(partials, axis=0, dtype=np.float32)
    S = g_sum @ W2 + cnt[:, None] * b2[None, :]
    mean = S / np.maximum(cnt, 1.0)[:, None]
    z = np.maximum(mean @ fc1_W + fc1_b[None, :], 0.0)
    z = z @ fc2_W + fc2_b[None, :]
    zmax = z.max(axis=1, keepdims=True)
    lse = np.log(np.sum(np.exp(z - zmax), axis=1, keepdims=True)) + zmax
    return (z - lse).astype(np.float32)


_NC_CACHE = {}


def run_on_device(cfg, per_core, x, trace=False):
    key = (cfg.N, cfg.D, cfg.G, cfg.N_CORES, cfg.T_RUN, cfg.W)
    if key not in _NC_CACHE:
        _NC_CACHE[key] = build_kernel(cfg)
    nc = _NC_CACHE[key]
    import ml_dtypes
    xp = pack_x(x)
    b1z = np.zeros((1, cfg.D), dtype=np.float32)
    in_maps = []
    for c in range(cfg.N_CORES):
        s = per_core[c]
        xsh = np.zeros((cfg.SHARD_PAD, cfg.D), dtype=xp.dtype)
        xsh[:cfg.SHARD] = xp[c * cfg.SHARD:(c + 1) * cfg.SHARD]
        in_maps.append({
            "x": xp, "xsh": xsh, "gidx": s["gidx"], "oh": s["oh"],
            "aggX": s["aggX"].astype(ml_dtypes.bfloat16), "dg": s["dg"],
            "wT": s["wT"], "W1": None, "b1": b1z,
        })
    return nc, in_maps


def kernel(x, edge_index, batch, W1, b1, W2, b2, fc1_W, fc1_b, fc2_W, fc2_b):
    from concourse import bass_utils
    cfg = FULL
    per_core, cnt, _ = preprocess(cfg, x, edge_index, batch)
    nc, in_maps = run_on_device(cfg, per_core, x)
    W1f = np.ascontiguousarray(np.asarray(W1, dtype=np.float32))
    b1f = np.asarray(b1, dtype=np.float32).reshape(1, cfg.D)
    for m in in_maps:
        m["W1"] = W1f
        m["b1"] = b1f
    res = bass_utils.run_bass_kernel_spmd(
        nc, in_maps, core_ids=list(range(cfg.N_CORES)))
    partials = [res.results[c]["partial"] for c in range(cfg.N_CORES)]
    out = epilogue(partials, cnt,
                   np.asarray(W2, np.float32), np.asarray(b2, np.float32),
                   np.asarray(fc1_W, np.float32), np.asarray(fc1_b, np.float32),
                   np.asarray(fc2_W, np.float32), np.asarray(fc2_b, np.float32))
    return out


# revision 16
# speedup vs baseline: 1.0617x; 1.0010x over previous
"""Trainium2 Bass kernel for a 2-layer GCN + global mean pool + MLP head.

Strategy (8 NeuronCores, SPMD, one shared NEFF):
  - Nodes (= aggregation dsts) are sharded across cores: core c owns rows
    [c*12500, (c+1)*12500), padded to 12800 = 25 blocks of 512.
  - Layer 1 is computed as agg = A_norm @ x (gather + one-hot-matmul
    segment-sum on device), then h1e = elu(agg.T @ W1 + b1) per shard.
  - Layer 2 + global mean pool collapse algebraically: the pooled sums are
    (B^T A_norm) h1e W2 with B the graph one-hot; wmat = B^T A_norm is a
    [64 x N] matrix computed on host from edge_index alone (graph
    partitioning / preprocessing, per the sharding hint). Each core
    computes partial = wmat[:, shard].T-contraction with h1e on device.
  - Host epilogue: sum 8 partials, @W2, mean, fc1/relu/fc2, log_softmax
    (~0.003% of FLOPs).

  Non-loop edges are routed on host to fixed "slot tiles":
  run = (dst block of 512, src chunk of 25000), T_RUN=15 tiles of 128 edge
  slots each; tile j accumulates into PSUM columns [WBASE[j], WBASE[j]+W)
  (W=192) of its block. The gathered 128-edge tile (bf16 rows of x, one 256B
  descriptor per edge) is the matmul stationary operand; the moving operand
  is a host-precomputed sparse routing tile oh[slot, w] = norm_e for the
  edge in that slot (norm folded in, zero elsewhere), streamed as bf16.
  All instruction structure (incl. PSUM offsets) is input-independent, so
  the NEFF is reusable for any same-shape input.

  Self loops bypass the gather entirely: their contribution is
  diag(dinv^2) applied to the core's own dense x shard, computed as NT
  small diagonal matmuls per block (xsh is a per-core sharded input).

  Overflow valve: edges that don't fit their run's slots (~6% at ~104%
  mean slot demand) are pre-aggregated on host into aggX, which also
  serves as the PSUM initializer via an identity-matmul.

  Perf note: the kernel is bound by SWDGE descriptor generation for the
  edge gather (~8 ns/descriptor on GpSimd Q7 cores 0/1, measured; engine-
  serial). DMA, TensorE and DVE all hide underneath it. dma_gather is the
  fastest indexed-move primitive on TRN2 (ap_gather measured ~33 ns/idx,
  single_packet=True and fp32 gathers hang the device, host-side
  descriptor generation only exists for compile-time-static patterns), so
  minimizing descriptor count (wide 128-dst windows, self-loop bypass,
  tight 97.7% slot packing) is the whole game.
"""
import sys
import numpy as np

sys.path.insert(0, "/opt/trn_rl_repo")


# ---------------------------------------------------------------- config ----
class CFG:
    def __init__(self, N=100000, D=128, G=64, n_cores=8, n_chunk=4, blk=512,
                 t_run=15, w=192, stride=23):
        self.N, self.D, self.G = N, D, G
        self.N_CORES, self.N_CHUNK, self.BLK = n_cores, n_chunk, blk
        self.T_RUN, self.W, self.STRIDE = t_run, w, stride
        assert N % n_cores == 0 and N % n_chunk == 0
        self.SHARD = N // n_cores
        self.CHUNK = N // n_chunk
        assert self.CHUNK <= 32768, "src chunk ids must fit int16"
        self.N_BLK = (self.SHARD + blk - 1) // blk
        self.SHARD_PAD = self.N_BLK * blk
        self.WBASE = np.minimum(np.arange(t_run) * stride, blk - w)
        assert self.WBASE[-1] == blk - w, "windows must cover the block"
        assert np.all(np.diff(self.WBASE) <= w)
        self.SLOTS = t_run * 128
        self.N_RUN = self.N_BLK * n_chunk
        self.NT = blk // 128  # node tiles per block


FULL = CFG()


# ---------------------------------------------------------- preprocessing ----
def preprocess(cfg, x, edge_index, batch):
    """Host-side graph partitioning: per-core gather/routing streams."""
    src = np.asarray(edge_index[0], dtype=np.int64)
    dst = np.asarray(edge_index[1], dtype=np.int64)
    batch = np.asarray(batch, dtype=np.int64)
    N, G = cfg.N, cfg.G

    deg = np.bincount(dst, minlength=N).astype(np.float32) + 1.0  # + self loop
    dinv = (1.0 / np.sqrt(deg)).astype(np.float32)

    loops = np.arange(N, dtype=np.int64)
    src_wm = np.concatenate([src, loops])
    dst_wm = np.concatenate([dst, loops])
    norm_wm = (dinv[src_wm] * dinv[dst_wm]).astype(np.float32)

    flat = batch[dst_wm] * N + src_wm
    wmat = np.bincount(flat, weights=norm_wm.astype(np.float64),
                       minlength=G * N).reshape(G, N).astype(np.float32)

    # self loops bypass the gather (handled by diag matmuls on dense x)
    src_all, dst_all, norm_all = src, dst, norm_wm[:len(src)]
    dl2 = (dinv * dinv).astype(np.float32)  # self-loop weights

    wb, W, T_RUN, BLK = cfg.WBASE, cfg.W, cfg.T_RUN, cfg.BLK
    per_core = []
    n_overflow = 0
    for c in range(cfg.N_CORES):
        m = (dst_all >= c * cfg.SHARD) & (dst_all < (c + 1) * cfg.SHARD)
        s_c = src_all[m]
        dl_c = dst_all[m] - c * cfg.SHARD
        nv_c = norm_all[m]
        blk = dl_c // BLK
        chunk = s_c // cfg.CHUNK
        o = np.lexsort((dl_c, chunk, blk))
        s_c, dl_c, nv_c, blk, chunk = s_c[o], dl_c[o], nv_c[o], blk[o], chunk[o]

        gidx = np.zeros((cfg.N_RUN, cfg.SLOTS), dtype=np.int16)
        # routing one-hot with norm folded in: [run, slot, W]
        ohv = np.zeros((cfg.N_RUN, cfg.SLOTS, W), dtype=np.float32)
        aggX = np.zeros((cfg.SHARD_PAD, cfg.D), dtype=np.float32)
        ovf_d, ovf_s, ovf_n = [], [], []

        run_id = blk * cfg.N_CHUNK + chunk
        run_starts = np.searchsorted(run_id, np.arange(cfg.N_RUN))
        run_ends = np.searchsorted(run_id, np.arange(cfg.N_RUN) + 1)
        for r in range(cfg.N_RUN):
            a, b = int(run_starts[r]), int(run_ends[r])
            if a == b:
                continue
            bb = (r // cfg.N_CHUNK) * BLK
            ch = r % cfg.N_CHUNK
            drel = dl_c[a:b] - bb
            # exact greedy: leftmost eligible tile, ascending dst
            starts = np.searchsorted(drel, wb)
            ends = np.searchsorted(drel, wb + W)
            tile_of = np.full(b - a, -1, dtype=np.int64)
            placed = 0
            for j in range(T_RUN):
                lo = max(placed, int(starts[j]))
                take = min(128, int(ends[j]) - lo)
                if take > 0:
                    tile_of[lo:lo + take] = j
                    placed = lo + take
                else:
                    placed = max(placed, lo)
            okm = tile_of >= 0
            tloc = tile_of[okm]
            loads = np.bincount(tloc, minlength=T_RUN)
            cum = np.concatenate([[0], np.cumsum(loads)])[:-1]
            pos = np.arange(len(tloc)) - np.repeat(cum, loads)
            slot = tloc * 128 + pos
            gidx[r, slot] = (s_c[a:b][okm] - ch * cfg.CHUNK).astype(np.int16)
            ohv[r, slot, (drel[okm] - wb[tloc]).astype(np.int64)] = nv_c[a:b][okm]
            ovf = np.nonzero(~okm)[0]
            if len(ovf):
                n_overflow += len(ovf)
                ovf_d.append(dl_c[a + ovf])
                ovf_s.append(s_c[a + ovf])
                ovf_n.append(nv_c[a + ovf])

        if ovf_d:
            od = np.concatenate(ovf_d)
            osrc = np.concatenate(ovf_s)
            on = np.concatenate(ovf_n)
            np.add.at(aggX, od,
                      on[:, None] * np.asarray(x, dtype=np.float32)[osrc])

        # self-loop diag tiles: dg[b, p, nt, q] = dl2[node] at p == q
        sh0 = c * cfg.SHARD
        dlpad = np.zeros(cfg.SHARD_PAD, dtype=np.float32)
        dlpad[:cfg.SHARD] = dl2[sh0:sh0 + cfg.SHARD]
        dg = np.zeros((cfg.N_BLK, 128, cfg.NT, 128), dtype=np.float32)
        ar = np.arange(128)
        dgv = dlpad.reshape(cfg.N_BLK, cfg.NT, 128)
        for b in range(cfg.N_BLK):
            for nt in range(cfg.NT):
                dg[b, ar, nt, ar] = dgv[b, nt]

        wT = np.zeros((cfg.SHARD_PAD, G), dtype=np.float32)
        wT[:cfg.SHARD] = wmat[:, c * cfg.SHARD:(c + 1) * cfg.SHARD].T

        import ml_dtypes
        per_core.append({
            # wrapped [16, SLOTS//16] then replicated to 128 partitions
            # (the SWDGE desc-gen cores read 16-partition wrapped indices)
            "gidx": np.ascontiguousarray(np.tile(
                gidx.reshape(cfg.N_RUN, cfg.SLOTS // 16, 16)
                    .transpose(0, 2, 1), (1, 8, 1))),
            # [run, 128 slot-partitions, T, W]
            "oh": np.ascontiguousarray(
                ohv.reshape(cfg.N_RUN, T_RUN, 128, W).transpose(0, 2, 1, 3)
            ).astype(ml_dtypes.bfloat16),
            "aggX": np.ascontiguousarray(aggX.T),      # [D, SHARD_PAD]
            "dg": dg.astype(ml_dtypes.bfloat16),        # [N_BLK, 128, NT, 128]
            "wT": wT,                                   # [SHARD_PAD, G]
        })

    cnt = np.bincount(batch, minlength=G).astype(np.float32)
    return per_core, cnt, n_overflow


def pack_x(x):
    """[N, D] f32 -> [N, D] bf16."""
    import ml_dtypes
    return np.ascontiguousarray(
        np.asarray(x, dtype=np.float32).astype(ml_dtypes.bfloat16))


# ---------------------------------------------------------- bass kernel ----
def build_kernel(cfg):
    from concourse import bacc, bass, tile, mybir
    from concourse.masks import make_identity
    f32 = mybir.dt.float32
    bf16 = mybir.dt.bfloat16

    nc = bacc.Bacc("TRN2", target_bir_lowering=False, debug=False,
                   enable_asserts=False)
    x_t = nc.dram_tensor("x", [cfg.N, cfg.D], bf16, kind="ExternalInput")
    xsh_t = nc.dram_tensor("xsh", [cfg.SHARD_PAD, cfg.D], bf16,
                           kind="ExternalInput")
    gidx_t = nc.dram_tensor("gidx", [cfg.N_RUN, 128, cfg.SLOTS // 16],
                            mybir.dt.int16, kind="ExternalInput")
    oh_t = nc.dram_tensor("oh", [cfg.N_RUN, 128, cfg.T_RUN, cfg.W], bf16,
                          kind="ExternalInput")
    aggx_t = nc.dram_tensor("aggX", [cfg.D, cfg.SHARD_PAD], bf16,
                            kind="ExternalInput")
    dg_t = nc.dram_tensor("dg", [cfg.N_BLK, 128, cfg.NT, 128], bf16,
                          kind="ExternalInput")
    wT_t = nc.dram_tensor("wT", [cfg.SHARD_PAD, cfg.G], f32,
                          kind="ExternalInput")
    W1_t = nc.dram_tensor("W1", [cfg.D, cfg.D], f32, kind="ExternalInput")
    b1_t = nc.dram_tensor("b1", [1, cfg.D], f32, kind="ExternalInput")
    out_t = nc.dram_tensor("partial", [cfg.G, cfg.D], f32,
                           kind="ExternalOutput")

    T, W, NT, BLK = cfg.T_RUN, cfg.W, cfg.NT, cfg.BLK
    WB = [int(v) for v in cfg.WBASE]
    mult, add, amax = (mybir.AluOpType.mult, mybir.AluOpType.add,
                       mybir.AluOpType.max)

    with tile.TileContext(nc) as tc:
        with (tc.tile_pool(name="const", bufs=1) as cpool,
              tc.tile_pool(name="gbuf", bufs=4) as gpool,
              tc.tile_pool(name="meta", bufs=6) as mpool,
              tc.tile_pool(name="oh", bufs=4) as opool,
              tc.tile_pool(name="agg", bufs=2) as apool,
              tc.tile_pool(name="eluv", bufs=2) as epool,
              tc.tile_pool(name="wt", bufs=2) as wpool,
              tc.tile_pool(name="ax", bufs=2) as axpool,
              tc.tile_pool(name="psA", bufs=2, space="PSUM") as pApool,
              tc.tile_pool(name="psB", bufs=2, space="PSUM") as pBpool,
              tc.tile_pool(name="psC", bufs=1, space="PSUM") as pCpool,
              tc.tile_pool(name="outp", bufs=1) as outpool):

            ident = cpool.tile([128, 128], bf16)
            make_identity(nc, ident[:])
            W1s = cpool.tile([128, cfg.D], f32)
            nc.sync.dma_start(W1s[:], W1_t.ap())
            b1s = cpool.tile([128, cfg.D], f32)
            nc.sync.dma_start(b1s[:], b1_t.ap().to_broadcast((128, cfg.D)))

            psC = pCpool.tile([cfg.G, cfg.D], f32)

            for b in range(cfg.N_BLK):
                axs = axpool.tile([128, BLK], bf16)
                nc.sync.dma_start(axs[:], aggx_t.ap()[:, b * BLK:(b + 1) * BLK])
                psA = pApool.tile([128, BLK], f32)
                nc.tensor.matmul(out=psA[:], lhsT=ident[:], rhs=axs[:],
                                 start=True, stop=False)

                # self loops: psA[:, g] += xsh_g.T @ diag(dinv^2)
                xds = axpool.tile([128, cfg.NT, cfg.D], bf16, tag="xd")
                nc.sync.dma_start(
                    xds[:],
                    xsh_t.ap()[b * BLK:(b + 1) * BLK, :]
                        .rearrange("(t p) d -> p t d", p=128))
                dgs = axpool.tile([128, cfg.NT, 128], bf16, tag="dg")
                nc.sync.dma_start(dgs[:], dg_t.ap()[b])
                for nt in range(NT):
                    nc.tensor.matmul(
                        out=psA[:, nt * 128:(nt + 1) * 128],
                        lhsT=xds[:, nt, :], rhs=dgs[:, nt, :],
                        start=False, stop=False)
                for ch in range(cfg.N_CHUNK):
                    r = b * cfg.N_CHUNK + ch
                    gxs = mpool.tile([128, cfg.SLOTS // 16], mybir.dt.int16,
                                     tag="gx")
                    nc.sync.dma_start(gxs[:], gidx_t.ap()[r])
                    oh = opool.tile([128, T, W], bf16)
                    nc.sync.dma_start(oh[:], oh_t.ap()[r])

                    gb = gpool.tile([128, T, cfg.D], bf16)
                    nc.gpsimd.dma_gather(
                        out_ap=gb[:],
                        in_ap=x_t.ap()[ch * cfg.CHUNK:(ch + 1) * cfg.CHUNK, :],
                        idxs_ap=gxs[:],
                        num_idxs=cfg.SLOTS,
                        num_idxs_reg=cfg.SLOTS,
                        elem_size=cfg.D,
                        single_packet=False,
                    )

                    for t in range(T):
                        last = (ch == cfg.N_CHUNK - 1 and t == T - 1)
                        nc.tensor.matmul(
                            out=psA[:, WB[t]:WB[t] + W],
                            lhsT=gb[:, t, :],
                            rhs=oh[:, t, :],
                            start=False, stop=last,
                        )

                # drain agg (feat-major [D x BLK])
                aggs = apool.tile([128, BLK], f32)
                nc.vector.tensor_copy(out=aggs[:], in_=psA[:])

                # B: h1 = agg.T @ W1  -> psB [node x feat_out], per node tile
                psB = pBpool.tile([128, BLK], f32)
                for nt in range(NT):
                    nc.tensor.matmul(out=psB[:, nt * cfg.D:(nt + 1) * cfg.D],
                                     lhsT=aggs[:, nt * 128:(nt + 1) * 128],
                                     rhs=W1s[:], start=True, stop=True)

                # elu(x+b1) = max(xb, min(exp(xb)-1, 0))
                xb = epool.tile([128, NT, cfg.D], f32, tag="xb")
                nc.vector.tensor_tensor(
                    out=xb[:],
                    in0=psB[:].rearrange("p (t d) -> p t d", d=cfg.D),
                    in1=b1s[:].unsqueeze(1).broadcast_to((128, NT, cfg.D)),
                    op=add)
                ex = epool.tile([128, NT * cfg.D], f32, tag="ex")
                nc.scalar.activation(
                    out=ex[:], in_=xb[:].rearrange("p t d -> p (t d)"),
                    func=mybir.ActivationFunctionType.Exp)
                nc.vector.tensor_scalar(
                    out=ex[:], in0=ex[:], scalar1=-1.0, scalar2=0.0,
                    op0=add, op1=mybir.AluOpType.min)
                h1e = epool.tile([128, NT * cfg.D], f32, tag="h1e")
                nc.vector.tensor_tensor(
                    out=h1e[:], in0=xb[:].rearrange("p t d -> p (t d)"),
                    in1=ex[:], op=amax)

                # C: partial += wT_block.T @ h1e
                wts = wpool.tile([128, NT, cfg.G], f32)
                nc.sync.dma_start(
                    wts[:],
                    wT_t.ap()[b * BLK:(b + 1) * BLK, :]
                        .rearrange("(t p) g -> p t g", p=128))
                for nt in range(NT):
                    nc.tensor.matmul(
                        out=psC[:],
                        lhsT=wts[:, nt, :],
                        rhs=h1e[:, nt * cfg.D:(nt + 1) * cfg.D],
                        start=(b == 0 and nt == 0),
                        stop=(b == cfg.N_BLK - 1 and nt == NT - 1),
                    )

            outs = outpool.tile([cfg.G, cfg.D], f32)
            nc.vector.tensor_copy(out=outs[:], in_=psC[:])
            nc.sync.dma_start(out_t.ap(), outs[:])

    nc.compile()
    return nc


# ------------------------------------------------------------- epilogue ----
def epilogue(partials, cnt, W2, b2, fc1_W, fc1_b, fc2_W, fc2_b):
    g_sum = np.sum(partials, axis=0, dtype=np.float32)
    S = g_sum @ W2 + cnt[:, None] * b2[None, :]
    mean = S / np.maximum(cnt, 1.0)[:, None]
    z = np.maximum(mean @ fc1_W + fc1_b[None, :], 0.0)
    z = z @ fc2_W + fc2_b[None, :]
    zmax = z.max(axis=1, keepdims=True)
    lse = np.log(np.sum(np.exp(z - zmax), axis=1, keepdims=True)) + zmax
    return (z - lse).astype(np.float32)


_NC_CACHE = {}


def run_on_device(cfg, per_core, x, trace=False):
    key = (cfg.N, cfg.D, cfg.G, cfg.N_CORES, cfg.T_RUN, cfg.W)
    if key not in _NC_CACHE:
        _NC_CACHE[key] = build_kernel(cfg)
    nc = _NC_CACHE[key]
    import ml_dtypes
    xp = pack_x(x)
    b1z = np.zeros((1, cfg.D), dtype=np.float32)
    in_maps = []
    for c in range(cfg.N_CORES):
        s = per_core[c]
        xsh = np.zeros((cfg.SHARD_PAD, cfg.D), dtype=xp.dtype)
        xsh[:cfg.SHARD] = xp[c * cfg.SHARD:(c + 1) * cfg.SHARD]
        in_maps.append({
            "x": xp, "xsh": xsh, "gidx": s["gidx"], "oh": s["oh"],
            "aggX": s["aggX"].astype(ml_dtypes.bfloat16), "dg": s["dg"],
            "wT": s["wT"], "W1": None, "b1": b1z,
        })
    return nc, in_maps


def kernel(x, edge_index, batch, W1, b1, W2, b2, fc1_W, fc1_b, fc2_W, fc2_b):
    from concourse import bass_utils
    cfg = FULL
    per_core, cnt, _ = preprocess(cfg, x, edge_index, batch)
    nc, in_maps = run_on_device(cfg, per_core, x)
    W1f = np.ascontiguousarray(np.asarray(W1, dtype=np.float32))
    b1f = np.asarray(b1, dtype=np.float32).reshape(1, cfg.D)
    for m in in_maps:
        m["W1"] = W1f
        m["b1"] = b1f
    res = bass_utils.run_bass_kernel_spmd(
        nc, in_maps, core_ids=list(range(cfg.N_CORES)))
    partials = [res.results[c]["partial"] for c in range(cfg.N_CORES)]
    out = epilogue(partials, cnt,
                   np.asarray(W2, np.float32), np.asarray(b2, np.float32),
                   np.asarray(fc1_W, np.float32), np.asarray(fc1_b, np.float32),
                   np.asarray(fc2_W, np.float32), np.asarray(fc2_b, np.float32))
    return out
